# revision 1
# baseline (speedup 1.0000x reference)
"""Distributed Trainium2 Bass kernel for nn_Attention_74732430950409.

Single-query MHA with RoPE'd keys/values, 4 projection weights folded
algebraically onto the tiny query side:

  qtil[h,:] = (((x @ Wq.T) @ Wq_mha.T)[h] @ Wk_mha[h]) @ Wk        (16, 2048)
  logits[s,h] = rope(keys)[s,:] . qtil[h,:] / sqrt(128)
  w = exp(logits)          (no max subtraction; |logits| < ~6)
  u[h,:] = sum_s w[s,h] * rope(states)[s,:]                        (16, 2048)
  l[h]   = sum_s w[s,h]
  z[h,:]  = (u[h,:] @ Wv.T) / l[h]                                 (16, 2048)
  attn[h,:] = z[h,:] @ Wv_mha[h].T                                 (16, 128)
  out = attn.flat @ Wo.T + x

Sequence-sharded across 8 cores (1024 rows each); all heavy tensors are
streamed once from HBM. Five collectives: AG(q), AG(tmp), AR(qtilT),
AR(u|l), AR(attnT). Compute dtype bf16 (f32 PSUM accum).
"""

import sys
import numpy as np

for p in ("/opt/trn_rl_repo",):
    if p not in sys.path:
        sys.path.insert(0, p)

import ml_dtypes

BF16 = ml_dtypes.bfloat16

NUM_HEADS = 16
QK = 2048
VO = 2048
S = 8192
NC = 8
S_LOC = S // NC          # 1024
SH = VO // NC            # 256 rows per core of each weight
DQ = QK // NUM_HEADS     # 128
HALF = VO // 2           # 1024
ROPE_THETA = 10000.0

_cache = {}


def _build():
    import concourse.bass as bass
    import concourse.mybir as mybir
    import concourse.bacc as bacc
    import concourse.tile as tile

    f32 = mybir.dt.float32
    bf16 = mybir.dt.bfloat16
    AF = mybir.ActivationFunctionType
    ALU = mybir.AluOpType
    PSUM = bass.MemorySpace.PSUM

    nc = bacc.Bacc(None, target_bir_lowering=False)

    # ---------------- DRAM parameters (per-core shards) ----------------
    keysT_d = nc.dram_tensor("keysT", [QK, S_LOC], bf16, kind="ExternalInput")
    states_d = nc.dram_tensor("states", [S_LOC, VO], bf16, kind="ExternalInput")
    xq_d = nc.dram_tensor("xq", [QK], bf16, kind="ExternalInput")
    identb_d = nc.dram_tensor("identb", [128, 128], bf16, kind="ExternalInput")
    xo_d = nc.dram_tensor("xo", [SH], f32, kind="ExternalInput")
    ident_d = nc.dram_tensor("ident", [128, 128], f32, kind="ExternalInput")
    ck_d = nc.dram_tensor("ck", [HALF, S_LOC], bf16, kind="ExternalInput")
    sk_d = nc.dram_tensor("sk", [HALF, S_LOC], bf16, kind="ExternalInput")
    cs_d = nc.dram_tensor("cs", [S_LOC, HALF], bf16, kind="ExternalInput")
    ss_d = nc.dram_tensor("ss", [S_LOC, HALF], bf16, kind="ExternalInput")
    wqT_d = nc.dram_tensor("wqT", [QK, SH], bf16, kind="ExternalInput")
    wqmC_d = nc.dram_tensor("wqmC", [SH, QK], bf16, kind="ExternalInput")
    wkmC_d = nc.dram_tensor("wkmC", [QK, SH], bf16, kind="ExternalInput")
    wk_d = nc.dram_tensor("wk", [SH, VO], bf16, kind="ExternalInput")
    wvT_d = nc.dram_tensor("wvT", [VO, SH], bf16, kind="ExternalInput")
    wvm_d = nc.dram_tensor("wvm", [SH, VO], bf16, kind="ExternalInput")
    woT_d = nc.dram_tensor("woT", [VO, SH], bf16, kind="ExternalInput")
    out_d = nc.dram_tensor("out", [1, SH], f32, kind="ExternalOutput")
    DEBUG = _cache.get("debug", False)
    if DEBUG:
        dqt_d = nc.dram_tensor("dbg_qt", [VO, NUM_HEADS], f32, kind="ExternalOutput")
        dw_d = nc.dram_tensor("dbg_w", [NUM_HEADS, S_LOC], f32, kind="ExternalOutput")
        du_d = nc.dram_tensor("dbg_u", [128, 16 * NUM_HEADS + 1], f32, kind="ExternalOutput")
        dat_d = nc.dram_tensor("dbg_at", [DQ, NUM_HEADS], f32, kind="ExternalOutput")
        dqh_d = nc.dram_tensor("dbg_qh", [1, QK], f32, kind="ExternalOutput")
        dtT_d = nc.dram_tensor("dbg_tT", [128, 2, NUM_HEADS], f32, kind="ExternalOutput")

    RG = [list(range(NC))]
    SCALE = 1.0 / float(np.sqrt(DQ))

    with tile.TileContext(nc) as tc:
        with (
            tc.tile_pool(name="kbuf", bufs=16) as kbuf,
            tc.tile_pool(name="sbuf_s", bufs=8) as sbuf_s,
            tc.tile_pool(name="tabs", bufs=1) as tabs,
            tc.tile_pool(name="wts", bufs=4) as wts,
            tc.tile_pool(name="tmps", bufs=8) as tmps,
            tc.tile_pool(name="small", bufs=1) as small,
            tc.tile_pool(name="psA", bufs=5, space=PSUM) as psA,
            tc.tile_pool(name="psB", bufs=3, space=PSUM) as psB,
            tc.tile_pool(name="dram", bufs=1, space="DRAM") as dram,
        ):
            # ---------------- collective bounce buffers ----------------
            bqh_in = dram.tile([128, NUM_HEADS], f32)
            bqh_out = dram.tile([128, NUM_HEADS], f32)
            bqt_in = dram.tile([128, 16 * NUM_HEADS], bf16)
            bqt_out = dram.tile([128, 16 * NUM_HEADS], bf16)
            bu_in = dram.tile([128, 16 * NUM_HEADS + 1], f32)
            bu_out = dram.tile([128, 16 * NUM_HEADS + 1], f32)
            bat_in = dram.tile([DQ, NUM_HEADS], f32)
            bqh = dram.tile([1, SH], f32)
            bat_out = dram.tile([DQ, NUM_HEADS], f32)


            # ---------------- small persistent SBUF tiles ----------------
            x_sb = small.tile([128, 16], bf16, tag="x")          # x as [k%128, kc]
            ident_f = small.tile([128, 128], f32, tag="idf")
            ident_b = small.tile([128, 128], bf16, tag="idb")
            qT_sb = small.tile([128, 2], bf16, tag="qT")         # local q shard, transposed
            qhT_sb = small.tile([128, 16], bf16, tag="qhT")      # full qh, transposed
            tmpT_sb = small.tile([128, 2, NUM_HEADS], bf16, tag="tmpT")
            qtp_sb = small.tile([128, 16, NUM_HEADS], bf16, tag="qtp")
            qtilT_sb = small.tile([128, 16, NUM_HEADS], bf16, tag="qtilT")
            w_sb = small.tile([NUM_HEADS, S_LOC], bf16, tag="w")
            l0_sb = small.tile([NUM_HEADS, 1], f32, tag="l0")
            l1_sb = small.tile([NUM_HEADS, 1], f32, tag="l1")
            lp_sb = small.tile([NUM_HEADS, 1], f32, tag="lp")
            wT_sb = small.tile([128, 8, NUM_HEADS], bf16, tag="wT")
            u_sb = small.tile([NUM_HEADS, VO], f32, tag="u")
            uT_sb = small.tile([128, 16, NUM_HEADS], f32, tag="uT")
            uT_bf = small.tile([128, 16, NUM_HEADS], bf16, tag="uTb")
            l_sb = small.tile([NUM_HEADS, 1], f32, tag="l")
            rl_sb = small.tile([NUM_HEADS, 1], f32, tag="rl")
            z_sb = small.tile([NUM_HEADS, SH], bf16, tag="z")
            zT_sb = small.tile([128, 2, NUM_HEADS], bf16, tag="zT")
            atT_sb = small.tile([128, NUM_HEADS], f32, tag="atT")
            atT_bf = small.tile([128, NUM_HEADS], bf16, tag="atTb")
            xo_sb = small.tile([1, SH], f32, tag="xo")
            out_sb = small.tile([1, SH], f32, tag="out")

            # ---------------- q-path weights ----------------
            wqT_sb = wts.tile([128, 16, SH], bf16, tag="w8k")
            wqmC_sb = wts.tile([128, 2, QK], bf16, tag="w8k")
            wkmC_sb = wts.tile([128, 16, SH], bf16, tag="w8k")
            wk_sb = wts.tile([128, 2, VO], bf16, tag="w8k")
            nc.sync.dma_start(wqT_sb[:], wqT_d[:, :].rearrange("(kc p) n -> p kc n", p=128))
            nc.sync.dma_start(wqmC_sb[:], wqmC_d[:, :].rearrange("(nc2 p) m -> p nc2 m", p=128))
            nc.sync.dma_start(wkmC_sb[:], wkmC_d[:, :].rearrange("(h p) j -> p h j", p=128))
            nc.sync.dma_start(wk_sb[:], wk_d[:, :].rearrange("(jc p) i -> p jc i", p=128))


            # x / identity
            nc.sync.dma_start(x_sb[:], xq_d[:].rearrange("(f p) -> p f", p=128))
            nc.sync.dma_start(ident_f[:], ident_d[:, :])
            nc.sync.dma_start(ident_b[:], identb_d[:, :])
            nc.sync.dma_start(xo_sb[:], xo_d[:].rearrange("(a n) -> a n", a=1))

            # ---------------- qT = (x @ Wq.T)^T  (local shard, [128, 2]) ----------------
            for nc2 in range(2):
                qt_ps2 = psB.tile([128, 1], f32, tag="pB", name=f"qt_ps2_{nc2}")
                for kc in range(16):
                    nc.tensor.matmul(qt_ps2[:], wqT_sb[:, kc, nc2 * 128 : (nc2 + 1) * 128],
                                     x_sb[:, kc : kc + 1], start=(kc == 0), stop=(kc == 15))
                nc.scalar.activation(qT_sb[:, nc2 : nc2 + 1], qt_ps2[:], AF.Copy)

            # ---------------- qhT partial [d, h] = (q_shard @ Wq_mha[:, shard].T)^T ----
            qhT_ps = psB.tile([128, NUM_HEADS], f32, tag="pB")
            for h in range(NUM_HEADS):
                for nc2 in range(2):
                    nc.tensor.matmul(qhT_ps[:, h : h + 1],
                                     wqmC_sb[:, nc2, h * 128 : (h + 1) * 128],
                                     qT_sb[:, nc2 : nc2 + 1],
                                     start=(nc2 == 0), stop=(nc2 == 1))
            qhTp_sb = small.tile([128, NUM_HEADS], f32, tag="qhTp")
            nc.scalar.activation(qhTp_sb[:], qhT_ps[:], AF.Copy)
            nc.sync.dma_start(bqh_in[:], qhTp_sb[:])
            nc.gpsimd.collective_compute(
                "AllReduce", ALU.add, ins=[bqh_in[:].opt()], outs=[bqh_out[:].opt()],
                replica_groups=RG)
            nc.gpsimd.dma_start(qhT_sb[:], bqh_out[:, :])

            # ---------------- tmpT[j, h] local j-shard ----------------
            tmpT_ps = [psB.tile([128, NUM_HEADS], f32, tag="pB", name=f"tmpT_ps{j}")
                       for j in range(2)]
            for h in range(NUM_HEADS):
                for jc in range(2):
                    nc.tensor.matmul(tmpT_ps[jc][:, h : h + 1],
                                     wkmC_sb[:, h, jc * 128 : (jc + 1) * 128],
                                     qhT_sb[:, h : h + 1], start=True, stop=True)
            for jc in range(2):
                nc.scalar.activation(tmpT_sb[:, jc, :], tmpT_ps[jc][:], AF.Copy)

            # ---------------- qtilT partial = Wk_shard.T-contract ----------------
            for ic in range(16):
                qt_ps = psB.tile([128, NUM_HEADS], f32, tag="pB")
                for jc in range(2):
                    nc.tensor.matmul(qt_ps[:], wk_sb[:, jc, ic * 128 : (ic + 1) * 128],
                                     tmpT_sb[:, jc, :], start=(jc == 0), stop=(jc == 1))
                nc.scalar.activation(qtp_sb[:, ic, :], qt_ps[:], AF.Copy)
            nc.sync.dma_start(bqt_in[:, :].rearrange("p (ic h) -> p ic h", ic=16), qtp_sb[:])
            nc.gpsimd.collective_compute(
                "AllReduce", ALU.add, ins=[bqt_in[:].opt()], outs=[bqt_out[:].opt()],
                replica_groups=RG)
            nc.gpsimd.dma_start(
                qtilT_sb[:], bqt_out[:, :].rearrange("p (ic h) -> p ic h", ic=16))

            # ---------------- stream keys (transposed layout) + tables ----------------
            ck_sb = tabs.tile([128, 8, S_LOC], bf16, tag="ck")
            sk_sb = tabs.tile([128, 8, S_LOC], bf16, tag="sk")
            nc.sync.dma_start(ck_sb[:], ck_d[:, :].rearrange("(t p) s -> p t s", p=128))
            nc.sync.dma_start(sk_sb[:], sk_d[:, :].rearrange("(t p) s -> p t s", p=128))

            kt = []
            for ci in range(16):
                t = kbuf.tile([128, S_LOC], bf16, tag="kt")
                nc.sync.dma_start(t[:], keysT_d[ci * 128 : (ci + 1) * 128, :])
                kt.append(t)

            # rope keys in place (pairs ci, ci+8)
            for ci in range(8):
                a, b = kt[ci], kt[ci + 8]
                t1 = tmps.tile([128, S_LOC], bf16, tag="rt")
                t2 = tmps.tile([128, S_LOC], bf16, tag="rt")
                t3 = tmps.tile([128, S_LOC], bf16, tag="rt")
                t4 = tmps.tile([128, S_LOC], bf16, tag="rt")
                nc.vector.tensor_mul(t1[:], a[:], ck_sb[:, ci, :])
                nc.vector.tensor_mul(t2[:], b[:], sk_sb[:, ci, :])
                nc.vector.tensor_mul(t3[:], b[:], ck_sb[:, ci, :])
                nc.vector.tensor_mul(t4[:], a[:], sk_sb[:, ci, :])
                nc.vector.tensor_sub(a[:], t1[:], t2[:])
                nc.vector.tensor_add(b[:], t3[:], t4[:])

            # ---------------- logits + exp ----------------
            for sc in range(2):
                lg_ps = psA.tile([NUM_HEADS, 512], f32, tag="pA")
                for ic in range(16):
                    nc.tensor.matmul(lg_ps[:], qtilT_sb[:, ic, :],
                                     kt[ic][:, sc * 512 : (sc + 1) * 512],
                                     start=(ic == 0), stop=(ic == 15))
                nc.scalar.activation(w_sb[:, sc * 512 : (sc + 1) * 512], lg_ps[:],
                                     AF.Exp, scale=SCALE,
                                     accum_out=(l0_sb[:] if sc == 0 else l1_sb[:]))
            nc.vector.tensor_add(lp_sb[:], l0_sb[:], l1_sb[:])

            # wT via PE transpose: [16,128] slices -> [128,16]
            for sb in range(8):
                tr_ps = psB.tile([128, NUM_HEADS], bf16, tag="pB")
                nc.tensor.transpose(tr_ps[:], w_sb[:, sb * 128 : (sb + 1) * 128],
                                    ident_b[0:NUM_HEADS, 0:NUM_HEADS])
                nc.scalar.activation(wT_sb[:, sb, :], tr_ps[:], AF.Copy)

            # ---------------- stream states + tables + rope ----------------
            cs_sb = tabs.tile([128, 8, HALF], bf16, tag="cs")
            ss_sb = tabs.tile([128, 8, HALF], bf16, tag="ss")
            nc.sync.dma_start(cs_sb[:], cs_d[:, :].rearrange("(t p) j -> p t j", p=128))
            nc.sync.dma_start(ss_sb[:], ss_d[:, :].rearrange("(t p) j -> p t j", p=128))

            st = []
            for sb in range(8):
                t = sbuf_s.tile([128, VO], bf16, tag="st")
                nc.sync.dma_start(t[:], states_d[sb * 128 : (sb + 1) * 128, :])
                st.append(t)

            for sb in range(8):
                t = st[sb]
                t1 = tmps.tile([128, HALF], bf16, tag="rt")
                t2 = tmps.tile([128, HALF], bf16, tag="rt")
                t3 = tmps.tile([128, HALF], bf16, tag="rt")
                t4 = tmps.tile([128, HALF], bf16, tag="rt")
                nc.vector.tensor_mul(t1[:], t[:, 0:HALF], cs_sb[:, sb, :])
                nc.vector.tensor_mul(t2[:], t[:, HALF:VO], ss_sb[:, sb, :])
                nc.vector.tensor_mul(t3[:], t[:, HALF:VO], cs_sb[:, sb, :])
                nc.vector.tensor_mul(t4[:], t[:, 0:HALF], ss_sb[:, sb, :])
                nc.vector.tensor_sub(t[:, 0:HALF], t1[:], t2[:])
                nc.vector.tensor_add(t[:, HALF:VO], t3[:], t4[:])

            # ---------------- u = wT.T @ states_pe ----------------
            u_ps = [psA.tile([NUM_HEADS, 512], f32, tag="pA", name=f"u_ps{i}")
                    for i in range(4)]
            for sb in range(8):
                for nch in range(4):
                    nc.tensor.matmul(u_ps[nch][:], wT_sb[:, sb, :],
                                     st[sb][:, nch * 512 : (nch + 1) * 512],
                                     start=(sb == 0), stop=(sb == 7))
            for nch in range(4):
                nc.scalar.activation(u_sb[:, nch * 512 : (nch + 1) * 512],
                                     u_ps[nch][:], AF.Copy)

            # uT via PE transpose (f32)
            for ic in range(16):
                tr_ps = psB.tile([128, NUM_HEADS], f32, tag="pB")
                nc.tensor.transpose(tr_ps[:], u_sb[:, ic * 128 : (ic + 1) * 128],
                                    ident_f[0:NUM_HEADS, 0:NUM_HEADS])
                nc.scalar.activation(uT_sb[:, ic, :], tr_ps[:], AF.Copy)
            nc.sync.dma_start(bu_in[:, 0:256].rearrange("p (ic h) -> p ic h", ic=16), uT_sb[:])
            nc.sync.dma_start(bu_in[0:NUM_HEADS, 256:257], lp_sb[:])
            nc.gpsimd.collective_compute(
                "AllReduce", ALU.add, ins=[bu_in[:].opt()], outs=[bu_out[:].opt()],
                replica_groups=RG)
            nc.gpsimd.dma_start(
                uT_bf[:], bu_out[:, 0:256].rearrange("p (ic h) -> p ic h", ic=16))
            nc.sync.dma_start(l_sb[:], bu_out[0:NUM_HEADS, 256:257])
            nc.vector.reciprocal(rl_sb[:], l_sb[:])

            # ---------------- epilogue weights ----------------
            wvT_sb = wts.tile([128, 16, SH], bf16, tag="w8k")
            wvm_sb = wts.tile([128, 2, VO], bf16, tag="w8k")
            woT_sb = wts.tile([128, 16, SH], bf16, tag="w8k")
            nc.sync.dma_start(wvT_sb[:], wvT_d[:, :].rearrange("(ic p) j -> p ic j", p=128))
            nc.sync.dma_start(wvm_sb[:], wvm_d[:, :].rearrange("(jc p) m -> p jc m", p=128))
            nc.sync.dma_start(woT_sb[:], woT_d[:, :].rearrange("(mc p) n -> p mc n", p=128))

            # ---------------- z = (u @ Wv.T) / l ----------------
            z_ps = psB.tile([NUM_HEADS, SH], f32, tag="pB")
            for ic in range(16):
                nc.tensor.matmul(z_ps[:], uT_bf[:, ic, :], wvT_sb[:, ic, :],
                                 start=(ic == 0), stop=(ic == 15))
            nc.scalar.activation(z_sb[:], z_ps[:], AF.Copy, scale=rl_sb[:])

            # zT
            for jc in range(2):
                tr_ps = psB.tile([128, NUM_HEADS], bf16, tag="pB")
                nc.tensor.transpose(tr_ps[:], z_sb[:, jc * 128 : (jc + 1) * 128],
                                    ident_b[0:NUM_HEADS, 0:NUM_HEADS])
                nc.scalar.activation(zT_sb[:, jc, :], tr_ps[:], AF.Copy)

            # ---------------- attn partial ----------------
            at_ps = psB.tile([128, NUM_HEADS], f32, tag="pB")
            for h in range(NUM_HEADS):
                for jc in range(2):
                    nc.tensor.matmul(at_ps[:, h : h + 1],
                                     wvm_sb[:, jc, h * 128 : (h + 1) * 128],
                                     zT_sb[:, jc, h : h + 1],
                                     start=(jc == 0), stop=(jc == 1))
            nc.scalar.activation(atT_sb[:], at_ps[:], AF.Copy)
            nc.sync.dma_start(bat_in[:], atT_sb[:])
            nc.gpsimd.collective_compute(
                "AllReduce", ALU.add, ins=[bat_in[:].opt()], outs=[bat_out[:].opt()],
                replica_groups=RG)
            nc.gpsimd.dma_start(atT_bf[:], bat_out[:, :])

            if DEBUG:
                nc.sync.dma_start(dqh_d[:, :], bqh_out[:, :])
                nc.gpsimd.dma_start(dtT_d[:, :, :], tmpT_sb[:])
                nc.sync.dma_start(dqt_d[:, :], bqt_out[:, :])
                nc.gpsimd.dma_start(dw_d[:, :], w_sb[:])
                nc.sync.dma_start(du_d[:, :], bu_out[:, :])
                nc.sync.dma_start(dat_d[:, :], bat_out[:, :])

            # ---------------- out = attn @ Wo.T + x ----------------
            o_ps = psB.tile([1, SH], f32, tag="pB")
            for h in range(NUM_HEADS):
                nc.tensor.matmul(o_ps[:], atT_bf[:, h : h + 1], woT_sb[:, h, :],
                                 start=(h == 0), stop=(h == NUM_HEADS - 1))
            nc.vector.tensor_add(out_sb[:], o_ps[:], xo_sb[:])
            nc.sync.dma_start(out_d[:, :], out_sb[:])

    nc.compile()
    return nc


def _tables():
    # mimic reference: f32 angles, f32 cos/sin, then bf16
    half = HALF
    freqs = 1.0 / (ROPE_THETA ** (np.arange(half, dtype=np.float32) * 2.0 / VO))
    ang = np.outer(np.arange(S, dtype=np.float32), freqs).astype(np.float32)  # (S, half)
    return np.cos(ang), np.sin(ang)


def kernel(x, keys, states, Wq, Wk, Wv, Wq_mha, Wk_mha, Wv_mha, Wo):
    from concourse import bass_utils

    if "nc" not in _cache:
        _cache["nc"] = _build()
    nc = _cache["nc"]

    x = np.asarray(x, np.float32)
    keys = np.asarray(keys, np.float32)
    states = np.asarray(states, np.float32)
    cos_t, sin_t = _tables()

    ident = np.eye(128, dtype=np.float32)
    in_maps = []
    for c in range(NC):
        rs = slice(c * SH, (c + 1) * SH)
        ss_ = slice(c * S_LOC, (c + 1) * S_LOC)
        cosc = cos_t[ss_]            # (1024, 1024) [s_loc, j]
        sinc = sin_t[ss_]
        m = {
            "keysT": np.ascontiguousarray(keys[ss_].T).astype(BF16),
            "states": np.ascontiguousarray(states[ss_]).astype(BF16),
            "xq": x.astype(BF16),
            "identb": ident.astype(BF16),
            "xo": np.ascontiguousarray(x[rs]),
            "ident": ident,
            "ck": np.ascontiguousarray(cosc.T).astype(BF16),
            "sk": np.ascontiguousarray(sinc.T).astype(BF16),
            "cs": np.ascontiguousarray(cosc).astype(BF16),
            "ss": np.ascontiguousarray(sinc).astype(BF16),
            "wqT": np.ascontiguousarray(Wq[rs].T).astype(BF16),
            "wqmC": np.ascontiguousarray(Wq_mha[:, rs].T).astype(BF16),
            "wkmC": np.ascontiguousarray(Wk_mha[:, rs]).astype(BF16),
            "wk": np.ascontiguousarray(Wk[rs]).astype(BF16),
            "wvT": np.ascontiguousarray(Wv[rs].T).astype(BF16),
            "wvm": np.ascontiguousarray(Wv_mha[:, rs].T).astype(BF16),
            "woT": np.ascontiguousarray(Wo[rs].T).astype(BF16),
        }
        in_maps.append(m)

    global _last_in_maps, _last_res
    _last_in_maps = in_maps
    res = bass_utils.run_bass_kernel_spmd(nc, in_maps, core_ids=list(range(NC)))
    _last_res = res
    out = np.concatenate([np.asarray(res.results[c]["out"]).reshape(-1) for c in range(NC)])
    return out[None, :].astype(np.float32)



# revision 8
# speedup vs baseline: 1.2902x; 1.2902x over previous
"""Distributed Trainium2 Bass kernel for nn_Attention_74732430950409.

Single-query MHA with RoPE'd keys/values; the four projections on the
query side are folded algebraically onto the (1 x d) query:

  qtil[h,:] = (((x @ Wq.T) @ Wq_mha.T)[h] @ Wk_mha[h]) @ Wk        (16, 2048)
  logits[s,h] = rope(keys)[s,:] . qtil[h,:] / sqrt(128)
  w = exp(logits)          (no max subtraction; |logits| small)
  u[h,:] = sum_s w[s,h] * rope(states)[s,:]                        (16, 2048)
  l[h]   = sum_s w[s,h]
  z[h,:]  = (u[h,:] @ Wv.T) / l[h]                                 (16, 2048)
  attn[h,:] = z[h,:] @ Wv_mha[h].T                                 (16, 128)
  out = attn.flat @ Wo.T + x

Sequence-sharded over 8 cores (1024 rows each); weights row-sharded
(256 rows each).  Four AllReduces: qh, qtilT, u|l, attnT.

Performance structure (vs the first working version):
  - All bulk input DMA rides the sync-engine HWDGE queue in strict
    priority order; host pre-tiles every tensor into [128, ...] layout
    so each descriptor is 2-8KB contiguous.
  - Collective bounce buffers + small reads use the gpsimd SW-DGE
    queue so AllReduces trigger as soon as their inputs exist instead
    of queueing behind the bulk stream.
  - RoPE is expressed as 4 elementwise products per tile pair; the
    combining add/sub is folded into the PE's PSUM accumulation of the
    logits / u GEMMs (negated copies of qtil / wT provide the signs).
  - States-side products are split across vector and gpsimd engines.
Compute dtype bf16 (f32 PSUM accumulation).
"""

import sys
import numpy as np

for p in ("/opt/trn_rl_repo",):
    if p not in sys.path:
        sys.path.insert(0, p)

import ml_dtypes

BF16 = ml_dtypes.bfloat16

NUM_HEADS = 16
QK = 2048
VO = 2048
S = 8192
NC = 8
S_LOC = S // NC          # 1024
SH = VO // NC            # 256 rows per core of each weight
DQ = QK // NUM_HEADS     # 128
HALF = VO // 2           # 1024
ROPE_THETA = 10000.0

_cache = {}


def _build():
    import concourse.bass as bass
    import concourse.mybir as mybir
    import concourse.bacc as bacc
    import concourse.tile as tile

    f32 = mybir.dt.float32
    bf16 = mybir.dt.bfloat16
    AF = mybir.ActivationFunctionType
    ALU = mybir.AluOpType
    PSUM = bass.MemorySpace.PSUM

    nc = bacc.Bacc(None, target_bir_lowering=False)

    # ---------------- DRAM parameters (per-core shards, pre-tiled) ----------
    # q-path weights
    wqT_d = nc.dram_tensor("wqT", [128, 16, SH], bf16, kind="ExternalInput")
    wqmC_d = nc.dram_tensor("wqmC", [128, 2, QK], bf16, kind="ExternalInput")
    wkmC_d = nc.dram_tensor("wkmC", [128, 16, SH], bf16, kind="ExternalInput")
    wk_d = nc.dram_tensor("wk", [128, 2, VO], bf16, kind="ExternalInput")
    # keys + k-layout tables
    ck_d = nc.dram_tensor("ck", [128, 8, S_LOC], bf16, kind="ExternalInput")
    sk_d = nc.dram_tensor("sk", [128, 8, S_LOC], bf16, kind="ExternalInput")
    keysT_d = nc.dram_tensor("keysT", [128, 8, 2, S_LOC], bf16, kind="ExternalInput")
    # states + s-layout tables
    cs_d = nc.dram_tensor("cs", [128, 8, HALF], bf16, kind="ExternalInput")
    ss_d = nc.dram_tensor("ss", [128, 8, HALF], bf16, kind="ExternalInput")
    states_d = nc.dram_tensor("states", [128, 8, VO], bf16, kind="ExternalInput")
    # epilogue weights
    wvT_d = nc.dram_tensor("wvT", [128, 16, SH], bf16, kind="ExternalInput")
    wvm_d = nc.dram_tensor("wvm", [128, 2, VO], bf16, kind="ExternalInput")
    woT_d = nc.dram_tensor("woT", [128, 16, SH], bf16, kind="ExternalInput")
    # small stuff
    xq_d = nc.dram_tensor("xq", [128, 16], bf16, kind="ExternalInput")
    ib16_d = nc.dram_tensor("ib16", [16, 16], bf16, kind="ExternalInput")
    ibn16_d = nc.dram_tensor("ibn16", [16, 16], bf16, kind="ExternalInput")
    if16_d = nc.dram_tensor("if16", [16, 16], f32, kind="ExternalInput")
    xo_d = nc.dram_tensor("xo", [1, SH], f32, kind="ExternalInput")
    out_d = nc.dram_tensor("out", [1, SH], f32, kind="ExternalOutput")
    DEBUG = _cache.get("debug", False)
    if DEBUG:
        dqt_d = nc.dram_tensor("dbg_qt", [128, 16 * NUM_HEADS], f32, kind="ExternalOutput")
        dw_d = nc.dram_tensor("dbg_w", [NUM_HEADS, S_LOC], f32, kind="ExternalOutput")
        du_d = nc.dram_tensor("dbg_u", [128, 16 * NUM_HEADS + 1], f32, kind="ExternalOutput")
        dat_d = nc.dram_tensor("dbg_at", [DQ, NUM_HEADS], f32, kind="ExternalOutput")
        dz_d = nc.dram_tensor("dbg_z", [NUM_HEADS, SH], f32, kind="ExternalOutput")
        dwt_d = nc.dram_tensor("dbg_wt", [128, 8, NUM_HEADS], f32, kind="ExternalOutput")
        dwtn_d = nc.dram_tensor("dbg_wtn", [128, 8, NUM_HEADS], f32, kind="ExternalOutput")
        dqn_d = nc.dram_tensor("dbg_qn", [128, 16 * NUM_HEADS], f32, kind="ExternalOutput")

    RG = [list(range(NC))]
    SCALE = 1.0 / float(np.sqrt(DQ))

    with tile.TileContext(nc) as tc:
        with (
            tc.tile_pool(name="wts", bufs=4) as wts,
            tc.tile_pool(name="tabs", bufs=1) as tabs,
            tc.tile_pool(name="kbuf", bufs=3) as kbuf,
            tc.tile_pool(name="kp", bufs=20) as kp,
            tc.tile_pool(name="sbuf_s", bufs=2) as sbuf_s,
            tc.tile_pool(name="sp", bufs=14) as sp,
            tc.tile_pool(name="small", bufs=1) as small,
            tc.tile_pool(name="psL", bufs=2, space=PSUM) as psL,
            tc.tile_pool(name="psU", bufs=4, space=PSUM) as psU,
            tc.tile_pool(name="psS", bufs=2, space=PSUM) as psS,
            tc.tile_pool(name="dram", bufs=1, space="DRAM") as dram,
        ):
            # ---------------- collective bounce buffers (DRAM) --------------
            bqh_in = dram.tile([128, NUM_HEADS], f32)
            bqh_out = dram.tile([128, NUM_HEADS], f32)
            bqt_in = dram.tile([128, 16 * NUM_HEADS], bf16)
            bqt_out = dram.tile([128, 16 * NUM_HEADS], bf16)
            bu_in = dram.tile([128, 16 * NUM_HEADS + 1], f32)
            bu_out = dram.tile([128, 16 * NUM_HEADS + 1], f32)
            bat_in = dram.tile([DQ, NUM_HEADS], f32)
            bat_out = dram.tile([DQ, NUM_HEADS], f32)

            # ---------------- SBUF tiles ------------------------------------
            # q-path weights (rotate through 4 slots shared with epilogue wts)
            wqT_sb = wts.tile([128, 16, SH], bf16, tag="w8k")
            wqmC_sb = wts.tile([128, 2, QK], bf16, tag="w8k")
            wkmC_sb = wts.tile([128, 16, SH], bf16, tag="w8k")
            wk_sb = wts.tile([128, 2, VO], bf16, tag="w8k")

            ck_sb = tabs.tile([128, 8, S_LOC], bf16, tag="ck")
            sk_sb = tabs.tile([128, 8, S_LOC], bf16, tag="sk")
            cs_sb = tabs.tile([128, 8, HALF], bf16, tag="cs")
            ss_sb = tabs.tile([128, 8, HALF], bf16, tag="ss")

            x_sb = small.tile([128, 16], bf16, tag="x")
            ib16_sb = small.tile([16, 16], bf16, tag="ib16")
            ibn16_sb = small.tile([16, 16], bf16, tag="ibn16")
            if16_sb = small.tile([16, 16], f32, tag="if16")
            xo_sb = small.tile([1, SH], f32, tag="xo")

            qT_sb = small.tile([128, 2], bf16, tag="qT")
            qhTp_sb = small.tile([128, NUM_HEADS], f32, tag="qhTp")
            qhT_sb = small.tile([128, NUM_HEADS], bf16, tag="qhT")
            tmpT_sb = small.tile([128, 2, NUM_HEADS], bf16, tag="tmpT")
            qtp_sb = small.tile([128, 16, NUM_HEADS], bf16, tag="qtp")
            qtilT_sb = small.tile([128, 16, NUM_HEADS], bf16, tag="qtilT")
            qtilN_sb = small.tile([128, 16, NUM_HEADS], bf16, tag="qtilN")

            w_sb = small.tile([NUM_HEADS, S_LOC], bf16, tag="w")
            l0_sb = small.tile([NUM_HEADS, 1], f32, tag="l0")
            l1_sb = small.tile([NUM_HEADS, 1], f32, tag="l1")
            lp_sb = small.tile([NUM_HEADS, 1], f32, tag="lp")
            wT_sb = small.tile([128, 8, NUM_HEADS], bf16, tag="wT")
            wTn_sb = small.tile([128, 8, NUM_HEADS], bf16, tag="wTn")

            u_sb = small.tile([NUM_HEADS, VO], f32, tag="u")
            uT_sb = small.tile([128, 16, NUM_HEADS], f32, tag="uT")
            uT_bf = small.tile([128, 16, NUM_HEADS], bf16, tag="uTb")
            l_sb = small.tile([NUM_HEADS, 1], f32, tag="l")
            rl_sb = small.tile([NUM_HEADS, 1], f32, tag="rl")
            z_sb = small.tile([NUM_HEADS, SH], bf16, tag="z")
            zT_sb = small.tile([128, 2, NUM_HEADS], bf16, tag="zT")
            atT_sb = small.tile([128, NUM_HEADS], f32, tag="atT")
            atT_bf = small.tile([128, NUM_HEADS], bf16, tag="atTb")
            out_sb = small.tile([1, SH], f32, tag="out")

            # ================ sync-queue bulk DMA, strict priority ===========
            nc.sync.dma_start(wqT_sb[:], wqT_d[:, :, :])
            nc.sync.dma_start(wqmC_sb[:], wqmC_d[:, :, :])
            nc.sync.dma_start(wkmC_sb[:], wkmC_d[:, :, :])
            nc.sync.dma_start(wk_sb[:], wk_d[:, :, :])

            kt = []
            for ci in range(8):
                nc.sync.dma_start(ck_sb[:, ci, :], ck_d[:, ci, :])
                nc.sync.dma_start(sk_sb[:, ci, :], sk_d[:, ci, :])
                t = kbuf.tile([128, 2, S_LOC], bf16, tag="kt", name=f"kt{ci}")
                nc.sync.dma_start(t[:], keysT_d[:, ci, :, :])
                kt.append(t)

            st = []
            for sb in range(8):
                nc.sync.dma_start(cs_sb[:, sb, :], cs_d[:, sb, :])
                nc.sync.dma_start(ss_sb[:, sb, :], ss_d[:, sb, :])
                t = sbuf_s.tile([128, VO], bf16, tag="st", name=f"st{sb}")
                nc.sync.dma_start(t[:], states_d[:, sb, :])
                st.append(t)

            wvT_sb = wts.tile([128, 16, SH], bf16, tag="w8k")
            wvm_sb = wts.tile([128, 2, VO], bf16, tag="w8k")
            woT_sb = wts.tile([128, 16, SH], bf16, tag="w8k")
            nc.sync.dma_start(wvT_sb[:], wvT_d[:, :, :])
            nc.sync.dma_start(wvm_sb[:], wvm_d[:, :, :])
            nc.sync.dma_start(woT_sb[:], woT_d[:, :, :])

            # ================ scalar-queue small DMAs ========================
            nc.scalar.dma_start(x_sb[:], xq_d[:, :])
            nc.scalar.dma_start(ib16_sb[:], ib16_d[:, :])
            nc.scalar.dma_start(ibn16_sb[:], ibn16_d[:, :])
            nc.scalar.dma_start(if16_sb[:], if16_d[:, :])
            nc.scalar.dma_start(xo_sb[:], xo_d[:, :])

            # ================ q-path =========================================
            # qT = (x @ Wq.T)^T  (local output shard as [128, 2])
            for nc2 in range(2):
                qt_ps2 = psS.tile([128, 1], f32, tag="pS", name=f"qt_ps2_{nc2}")
                for kc in range(16):
                    nc.tensor.matmul(qt_ps2[:], wqT_sb[:, kc, nc2 * 128 : (nc2 + 1) * 128],
                                     x_sb[:, kc : kc + 1], start=(kc == 0), stop=(kc == 15))
                nc.scalar.activation(qT_sb[:, nc2 : nc2 + 1], qt_ps2[:], AF.Copy)

            # qhT partial [d, h] = (q_shard @ Wq_mha[:, shard].T)^T
            qhT_ps = psS.tile([128, NUM_HEADS], f32, tag="pS")
            for h in range(NUM_HEADS):
                for nc2 in range(2):
                    nc.tensor.matmul(qhT_ps[:, h : h + 1],
                                     wqmC_sb[:, nc2, h * 128 : (h + 1) * 128],
                                     qT_sb[:, nc2 : nc2 + 1],
                                     start=(nc2 == 0), stop=(nc2 == 1))
            nc.scalar.activation(qhTp_sb[:], qhT_ps[:], AF.Copy)
            nc.gpsimd.dma_start(bqh_in[:], qhTp_sb[:])
            nc.gpsimd.collective_compute(
                "AllReduce", ALU.add, ins=[bqh_in[:].opt()], outs=[bqh_out[:].opt()],
                replica_groups=RG)
            nc.gpsimd.dma_start(qhT_sb[:], bqh_out[:, :])

            # tmpT[j, h] local j-shard
            tmpT_ps = [psS.tile([128, NUM_HEADS], f32, tag="pS", name=f"tmpT_ps{j}")
                       for j in range(2)]
            for h in range(NUM_HEADS):
                for jc in range(2):
                    nc.tensor.matmul(tmpT_ps[jc][:, h : h + 1],
                                     wkmC_sb[:, h, jc * 128 : (jc + 1) * 128],
                                     qhT_sb[:, h : h + 1], start=True, stop=True)
            for jc in range(2):
                nc.scalar.activation(tmpT_sb[:, jc, :], tmpT_ps[jc][:], AF.Copy)

            # qtilT partial = Wk_shard.T-contract
            for ic in range(16):
                qt_ps = psS.tile([128, NUM_HEADS], f32, tag="pS")
                for jc in range(2):
                    nc.tensor.matmul(qt_ps[:], wk_sb[:, jc, ic * 128 : (ic + 1) * 128],
                                     tmpT_sb[:, jc, :], start=(jc == 0), stop=(jc == 1))
                nc.scalar.activation(qtp_sb[:, ic, :], qt_ps[:], AF.Copy)
            nc.gpsimd.dma_start(
                bqt_in[:, :].rearrange("p (ic h) -> p ic h", ic=16), qtp_sb[:])
            nc.gpsimd.collective_compute(
                "AllReduce", ALU.add, ins=[bqt_in[:].opt()], outs=[bqt_out[:].opt()],
                replica_groups=RG)
            nc.gpsimd.dma_start(
                qtilT_sb[:], bqt_out[:, :].rearrange("p (ic h) -> p ic h", ic=16))
            # negated copy (supplies the "-" of rope's first-half combine)
            nc.scalar.activation(qtilN_sb[:], qtilT_sb[:], AF.Copy, scale=-1.0)

            # ================ keys: products + logits ========================
            # pair ci covers j-chunks (ci, ci+8):
            #   a = keys[:, j=ci*128+p],  b = keys[:, j=1024+ci*128+p]
            #   roped_a = a*ck - b*sk   (lhsT qtil[ci] / qtilN[ci])
            #   roped_b = b*ck + a*sk   (lhsT qtil[ci+8])
            lg_ps = [psL.tile([NUM_HEADS, 512], f32, tag="pL", name=f"lg{sc}")
                     for sc in range(2)]
            for ci in range(8):
                a = kt[ci][:, 0, :]
                b = kt[ci][:, 1, :]
                t1 = kp.tile([128, S_LOC], bf16, tag="kp", name=f"t1_{ci}")
                t2 = kp.tile([128, S_LOC], bf16, tag="kp", name=f"t2_{ci}")
                t3 = kp.tile([128, S_LOC], bf16, tag="kp", name=f"t3_{ci}")
                t4 = kp.tile([128, S_LOC], bf16, tag="kp", name=f"t4_{ci}")
                nc.vector.tensor_mul(t1[:], a, ck_sb[:, ci, :])
                nc.vector.tensor_mul(t2[:], b, sk_sb[:, ci, :])
                nc.vector.tensor_mul(t3[:], b, ck_sb[:, ci, :])
                nc.vector.tensor_mul(t4[:], a, sk_sb[:, ci, :])
                prods = [(t1, qtilT_sb[:, ci, :]), (t2, qtilN_sb[:, ci, :]),
                         (t3, qtilT_sb[:, ci + 8, :]), (t4, qtilT_sb[:, ci + 8, :])]
                for pi, (t, lhsT) in enumerate(prods):
                    for sc in range(2):
                        nc.tensor.matmul(lg_ps[sc][:], lhsT,
                                         t[:, sc * 512 : (sc + 1) * 512],
                                         start=(ci == 0 and pi == 0),
                                         stop=(ci == 7 and pi == 3))

            # ================ softmax (no max-subtraction) ===================
            nc.scalar.activation(w_sb[:, 0:512], lg_ps[0][:], AF.Exp,
                                 scale=SCALE, accum_out=l0_sb[:])
            nc.scalar.activation(w_sb[:, 512:1024], lg_ps[1][:], AF.Exp,
                                 scale=SCALE, accum_out=l1_sb[:])

            # wT (+ negated) via PE transpose: [16,128] slices -> [128,16]
            for sb in range(8):
                tr_ps = psS.tile([128, NUM_HEADS], bf16, tag="pS")
                nc.tensor.transpose(tr_ps[:], w_sb[:, sb * 128 : (sb + 1) * 128],
                                    ib16_sb[:, :])
                nc.scalar.activation(wT_sb[:, sb, :], tr_ps[:], AF.Copy)
                nc.scalar.activation(wTn_sb[:, sb, :], tr_ps[:], AF.Copy, scale=-1.0)

            # ================ states: products + u ===========================
            #   st1 = states[:, 0:1024], st2 = states[:, 1024:2048]
            #   u_lo += wT.T@(st1*cs) + wTn.T@(st2*ss)
            #   u_hi += wT.T@(st2*cs) + wT.T @(st1*ss)
            u_ps = [psU.tile([NUM_HEADS, 512], f32, tag="pU", name=f"u_ps{i}")
                    for i in range(4)]
            for sb in range(8):
                st1 = st[sb][:, 0:HALF]
                st2 = st[sb][:, HALF:VO]
                p1 = sp.tile([128, HALF], bf16, tag="sp", name=f"p1_{sb}")
                p2 = sp.tile([128, HALF], bf16, tag="sp", name=f"p2_{sb}")
                p3 = sp.tile([128, HALF], bf16, tag="sp", name=f"p3_{sb}")
                p4 = sp.tile([128, HALF], bf16, tag="sp", name=f"p4_{sb}")
                nc.vector.tensor_mul(p1[:], st1, cs_sb[:, sb, :])
                nc.gpsimd.tensor_mul(p2[:], st2, ss_sb[:, sb, :])
                nc.vector.tensor_mul(p3[:], st2, cs_sb[:, sb, :])
                nc.vector.tensor_mul(p4[:], st1, ss_sb[:, sb, :])
                # banks 0/1 accumulate p1 then p2 per sb; banks 2/3 p3 then p4
                chunks = [(0, p1, wT_sb), (0, p2, wTn_sb), (2, p3, wT_sb), (2, p4, wT_sb)]
                for pi, (base, t, wtiles) in enumerate(chunks):
                    for nch in range(2):
                        nc.tensor.matmul(u_ps[base + nch][:], wtiles[:, sb, :],
                                         t[:, nch * 512 : (nch + 1) * 512],
                                         start=(sb == 0 and pi in (0, 2)),
                                         stop=(sb == 7 and pi in (1, 3)))

            # ================ u epilogue =====================================
            for nch in range(4):
                nc.scalar.activation(u_sb[:, nch * 512 : (nch + 1) * 512],
                                     u_ps[nch][:], AF.Copy)
            for ic in range(16):
                tr_ps = psS.tile([128, NUM_HEADS], f32, tag="pS")
                nc.tensor.transpose(tr_ps[:], u_sb[:, ic * 128 : (ic + 1) * 128],
                                    if16_sb[:, :])
                nc.scalar.activation(uT_sb[:, ic, :], tr_ps[:], AF.Copy)
            nc.vector.tensor_add(lp_sb[:], l0_sb[:], l1_sb[:])
            nc.gpsimd.dma_start(
                bu_in[:, 0:256].rearrange("p (ic h) -> p ic h", ic=16), uT_sb[:])
            nc.gpsimd.dma_start(bu_in[0:NUM_HEADS, 256:257], lp_sb[:])
            nc.gpsimd.collective_compute(
                "AllReduce", ALU.add, ins=[bu_in[:].opt()], outs=[bu_out[:].opt()],
                replica_groups=RG)
            nc.gpsimd.dma_start(
                uT_bf[:], bu_out[:, 0:256].rearrange("p (ic h) -> p ic h", ic=16))
            nc.gpsimd.dma_start(l_sb[:], bu_out[0:NUM_HEADS, 256:257])
            nc.vector.reciprocal(rl_sb[:], l_sb[:])

            # ================ tail: z, attn, out =============================
            z_ps = psS.tile([NUM_HEADS, SH], f32, tag="pS")
            for ic in range(16):
                nc.tensor.matmul(z_ps[:], uT_bf[:, ic, :], wvT_sb[:, ic, :],
                                 start=(ic == 0), stop=(ic == 15))
            nc.scalar.activation(z_sb[:], z_ps[:], AF.Copy, scale=rl_sb[:])

            for jc in range(2):
                tr_ps = psS.tile([128, NUM_HEADS], bf16, tag="pS")
                nc.tensor.transpose(tr_ps[:], z_sb[:, jc * 128 : (jc + 1) * 128],
                                    ib16_sb[:, :])
                nc.scalar.activation(zT_sb[:, jc, :], tr_ps[:], AF.Copy)

            at_ps = psS.tile([128, NUM_HEADS], f32, tag="pS")
            for h in range(NUM_HEADS):
                for jc in range(2):
                    nc.tensor.matmul(at_ps[:, h : h + 1],
                                     wvm_sb[:, jc, h * 128 : (h + 1) * 128],
                                     zT_sb[:, jc, h : h + 1],
                                     start=(jc == 0), stop=(jc == 1))
            nc.scalar.activation(atT_sb[:], at_ps[:], AF.Copy)
            nc.gpsimd.dma_start(bat_in[:], atT_sb[:])
            nc.gpsimd.collective_compute(
                "AllReduce", ALU.add, ins=[bat_in[:].opt()], outs=[bat_out[:].opt()],
                replica_groups=RG)
            nc.gpsimd.dma_start(atT_bf[:], bat_out[:, :])

            if DEBUG:
                nc.gpsimd.dma_start(dqt_d[:, :], bqt_out[:, :])
                nc.gpsimd.dma_start(dqn_d[:, :].rearrange("p (ic h) -> p ic h", ic=16), qtilN_sb[:])
                nc.gpsimd.dma_start(dw_d[:, :], w_sb[:])
                nc.gpsimd.dma_start(dwt_d[:, :, :], wT_sb[:])
                nc.gpsimd.dma_start(dwtn_d[:, :, :], wTn_sb[:])
                nc.gpsimd.dma_start(du_d[:, :], bu_out[:, :])
                nc.gpsimd.dma_start(dz_d[:, :], z_sb[:])
                nc.gpsimd.dma_start(dat_d[:, :], bat_out[:, :])

            o_ps = psS.tile([1, SH], f32, tag="pS")
            for h in range(NUM_HEADS):
                nc.tensor.matmul(o_ps[:], atT_bf[:, h : h + 1], woT_sb[:, h, :],
                                 start=(h == 0), stop=(h == NUM_HEADS - 1))
            nc.vector.tensor_add(out_sb[:], o_ps[:], xo_sb[:])
            nc.gpsimd.dma_start(out_d[:, :], out_sb[:])

    nc.compile()
    return nc


def _tables():
    # mimic reference: f32 angles, f32 cos/sin
    freqs = 1.0 / (ROPE_THETA ** (np.arange(HALF, dtype=np.float32) * 2.0 / VO))
    ang = np.outer(np.arange(S, dtype=np.float32), freqs).astype(np.float32)  # (S, half)
    return np.cos(ang), np.sin(ang)


def _tile_rows(a, p=128):
    """[R, C] -> [p, R//p, C] with row index = t*p + lane."""
    r, c = a.shape
    return np.ascontiguousarray(a.reshape(r // p, p, c).transpose(1, 0, 2))


def kernel(x, keys, states, Wq, Wk, Wv, Wq_mha, Wk_mha, Wv_mha, Wo):
    from concourse import bass_utils

    if "nc" not in _cache:
        _cache["nc"] = _build()
    nc = _cache["nc"]

    x = np.asarray(x, np.float32)
    keys = np.asarray(keys, np.float32)
    states = np.asarray(states, np.float32)
    cos_t, sin_t = _tables()

    ib = np.eye(16, dtype=np.float32)
    in_maps = []
    for c in range(NC):
        rs = slice(c * SH, (c + 1) * SH)
        sq = slice(c * S_LOC, (c + 1) * S_LOC)
        cosc = cos_t[sq]            # (1024, 1024) [s_loc, j]
        sinc = sin_t[sq]
        kT = keys[sq].T             # (2048, 1024) [j, s_loc]
        # pre-paired keys: [p, pair, half, s]
        kp = np.ascontiguousarray(
            kT.reshape(2, 8, 128, S_LOC).transpose(2, 1, 0, 3))
        m = {
            "wqT": _tile_rows(Wq[rs].T).astype(BF16),
            "wqmC": _tile_rows(Wq_mha[:, rs].T).astype(BF16),
            "wkmC": _tile_rows(Wk_mha[:, rs]).astype(BF16),
            "wk": _tile_rows(Wk[rs]).astype(BF16),
            "ck": _tile_rows(np.ascontiguousarray(cosc.T)).astype(BF16),
            "sk": _tile_rows(np.ascontiguousarray(sinc.T)).astype(BF16),
            "keysT": kp.astype(BF16),
            "cs": _tile_rows(cosc).astype(BF16),
            "ss": _tile_rows(sinc).astype(BF16),
            "states": _tile_rows(states[sq]).astype(BF16),
            "wvT": _tile_rows(Wv[rs].T).astype(BF16),
            "wvm": _tile_rows(Wv_mha[:, rs].T).astype(BF16),
            "woT": _tile_rows(Wo[rs].T).astype(BF16),
            "xq": np.ascontiguousarray(x.reshape(16, 128).T).astype(BF16),
            "ib16": ib.astype(BF16),
            "ibn16": (-ib).astype(BF16),
            "if16": ib,
            "xo": np.ascontiguousarray(x[rs])[None, :],
        }
        in_maps.append(m)

    global _last_in_maps, _last_res
    _last_in_maps = in_maps
    res = bass_utils.run_bass_kernel_spmd(nc, in_maps, core_ids=list(range(NC)))
    _last_res = res
    out = np.concatenate([np.asarray(res.results[c]["out"]).reshape(-1) for c in range(NC)])
    return out[None, :].astype(np.float32)
